# revision 43
# baseline (speedup 1.0000x reference)
"""
MultiHeadCrossAttention Trainium2 kernel (Bass/Tile), data-parallel over batch
on 8 NeuronCores.

Reference computation (per batch row b):
    Q = text @ Wq.T + bq          [B, 1024] -> [B, 8, 128]
    K = image @ Wk.T + bk         [B, 1024] -> [B, 8, 128]
    V = image @ Wv.T + bv         [B, 1024] -> [B, 8, 128]
    scores[b,h,g] = Q[b,h,:].K[b,g,:] / sqrt(128)
    attn = softmax_g(scores)
    attended[b,h,:] = sum_g attn[b,h,g] V[b,g,:]
    y = LayerNorm(text + attended) * gamma + beta

Design (per core, B_loc = 2048 batch rows):
  - Projections on the PE in fp16 (fp32 PSUM accumulation).  Stationary
    operand = X^T batch tile [128 d x 128 b], moving operand = W^T chunk
    [128 d x 512 f]; biases folded in with a K=1 ones-row matmul.
  - X^T comes from host-transposed DRAM copies; fp32->fp16 conversion is done
    by the cast-DMA (SWDGE), as is the weight load.
  - Attention entirely in batch-on-partition layout with 4 big DVE ops per
    tile: elementwise product + pool_avg for scores (one instruction each for
    all 64 (h,g) pairs via broadcast access patterns), ACT exp, then product +
    pool_avg again for the attend step.  The pool_avg 1/8 and 1/128 factors
    cancel exactly against the softmax normalization / score scale.
  - Residual + LayerNorm with bn_stats/bn_aggr + tensor_scalar; gamma/beta
    are pre-broadcast to [128, 1024] in SBUF.
"""

import functools
import sys

import numpy as np

sys.path.insert(0, "/opt/trn_rl_repo")

import concourse.bass as bass  # noqa: E402
import concourse.tile as tile  # noqa: E402
from concourse import bacc, bass_utils, mybir  # noqa: E402


def _patch_act_tables():
    """Force every activation we use (Exp/Ln/Square/Copy/Identity) to resolve
    to the one table set that holds them all (natural_log_exp_and_others), so
    bacc emits a single ACT table load instead of thrashing (1.28us/swap).
    Only the chooser's view is edited; set ids stay positional, so walrus
    still maps to the real act_info.json tables."""
    import concourse.hw_specs as hw_specs

    orig = hw_specs.get_activation_tables
    if getattr(orig, "_mhca_patched", False):
        return

    A = mybir.ActivationFunctionType
    KEEP = "natural_log_exp_and_others"

    @functools.cache
    def patched(arch):
        tabs = {k: set(v) for k, v in orig(arch).items()}
        for k, s in tabs.items():
            if k != KEEP:
                for f in (A.Exp, A.Ln, A.Square, A.Copy, A.Identity):
                    s.discard(f)
        return tabs

    patched._mhca_patched = True
    hw_specs.get_activation_tables = patched
    import concourse.bass_interp as _bi

    _bi.get_activation_tables = patched
    bacc.get_activation_tables = patched


_patch_act_tables()

# Problem constants (hardcoded per contest contract)
B = 16384
N_CORES = 8
B_LOC = B // N_CORES  # 2048
TEXT_DIM = 1024
IMAGE_DIM = 2048
H = 8
HD = 128
NTC = TEXT_DIM // 128  # 8 text d-chunks
NIC = IMAGE_DIM // 128  # 16 image d-chunks

BT = 128  # batch tile (partition dim)
NT = B_LOC // BT  # 16 batch tiles per core
PHASE = 2  # batch tiles per X^T slab load
BW = PHASE * BT  # slab batch width

F16 = mybir.dt.float16
F32 = mybir.dt.float32
F8 = mybir.dt.float8e4

INV_SQRT_HD = 1.0 / np.sqrt(128.0)
W_SCALE = 4096.0  # fp8 weight pre-scale (keeps w out of e4m3 subnormals)

# V feature permutation: f' = d*8 + g for original f = g*128 + d, i.e. V is
# stored with the 8 head values of each hidden position adjacent, so the
# attend product / g-reduction reads contiguous 8-element runs.
_d, _g = np.meshgrid(np.arange(128), np.arange(8), indexing="ij")
V_PERM = (_g * 128 + _d).reshape(-1)  # V_PERM[f'] = original f


def build_bass(b_loc: int = B_LOC, apply_affine: bool = False) -> bass.Bass:
    nt = b_loc // BT
    phase = min(PHASE, nt)
    bw = phase * BT

    nc = bacc.Bacc(trn_type="TRN2", debug=False, name="mhca_dp", num_swdge_queues=4)

    # ---- DRAM I/O ----
    text_t = nc.dram_tensor("text_t", [TEXT_DIM, b_loc], F8, kind="ExternalInput")
    image_t = nc.dram_tensor("image_t", [IMAGE_DIM, b_loc], F8, kind="ExternalInput")
    text = nc.dram_tensor("text", [b_loc, TEXT_DIM], F32, kind="ExternalInput")
    wq_t = nc.dram_tensor("wq_t", [TEXT_DIM, TEXT_DIM], F8, kind="ExternalInput")
    wk_t = nc.dram_tensor("wk_t", [IMAGE_DIM, TEXT_DIM], F8, kind="ExternalInput")
    wv_t = nc.dram_tensor("wv_t", [IMAGE_DIM, TEXT_DIM], F8, kind="ExternalInput")
    bq = nc.dram_tensor("bq", [1, TEXT_DIM], F16, kind="ExternalInput")
    bk = nc.dram_tensor("bk", [1, TEXT_DIM], F16, kind="ExternalInput")
    bv = nc.dram_tensor("bv", [1, TEXT_DIM], F16, kind="ExternalInput")
    gamma = nc.dram_tensor("gamma", [1, TEXT_DIM], F32, kind="ExternalInput")
    beta = nc.dram_tensor("beta", [1, TEXT_DIM], F32, kind="ExternalInput")
    y = nc.dram_tensor("y", [b_loc, TEXT_DIM], F32, kind="ExternalOutput")

    with tile.TileContext(nc) as tc:
        _body(nc, tc, locals(), nt=nt, phase=phase, bw=bw, apply_affine=apply_affine)
    nc.compile()
    return nc


def _ap(t: bass.AP, dims) -> bass.AP:
    """Raw AP on an SBUF tile: keep its partition dim, custom free dims."""
    return bass.AP(tensor=t.tensor, offset=t.offset, ap=[list(t.ap[0])] + [list(d) for d in dims])


def _body(
    nc: bass.Bass,
    tc: tile.TileContext,
    io: dict,
    *,
    nt: int,
    phase: int,
    bw: int,
    apply_affine: bool,
):
    text_t, image_t, text = io["text_t"], io["image_t"], io["text"]
    wq_t, wk_t, wv_t = io["wq_t"], io["wk_t"], io["wv_t"]
    bq, bk, bv = io["bq"], io["bk"], io["bv"]
    gamma, beta, y = io["gamma"], io["beta"], io["y"]

    import contextlib

    ctx = contextlib.ExitStack()
    with ctx:
        consts = ctx.enter_context(tc.tile_pool(name="consts", bufs=1))
        slabs = ctx.enter_context(tc.tile_pool(name="slabs", bufs=2))
        qkv = ctx.enter_context(tc.tile_pool(name="qkv", bufs=2))
        work = ctx.enter_context(tc.tile_pool(name="work", bufs=2))
        # paired prod/scr2 are single-buffered: the attend of pair p-1 is
        # emitted before stage_a(p) reuses them, so there is no WAR overlap
        prods = ctx.enter_context(tc.tile_pool(name="prods", bufs=1))
        scr2p = ctx.enter_context(tc.tile_pool(name="scr2p", bufs=1))
        outs = ctx.enter_context(tc.tile_pool(name="outs", bufs=2))
        small = ctx.enter_context(tc.tile_pool(name="small", bufs=3))
        psum = ctx.enter_context(tc.tile_pool(name="psum", bufs=8, space="PSUM"))

        # ---- constants / weights (fp8, host-quantized ×W_SCALE) ----
        w16_q = consts.tile([128, NTC, TEXT_DIM], F8)
        w16_k = consts.tile([128, NIC, TEXT_DIM], F8)
        w16_v = consts.tile([128, NIC, TEXT_DIM], F8)
        # chunked weight loads, interleaved in tile-0 consumption order so the
        # first matmuls can start long before all 20MB of weights land
        wq_r = wq_t[:].rearrange("(c p) f -> p c f", p=128)
        wk_r = wk_t[:].rearrange("(c p) f -> p c f", p=128)
        wv_r = wv_t[:].rearrange("(c p) f -> p c f", p=128)
        wload = [
            (w16_q, wq_r, 0), (w16_k, wk_r, 0),
            (w16_q, wq_r, 4), (w16_k, wk_r, 4),
            (w16_k, wk_r, 8), (w16_k, wk_r, 12),
            (w16_v, wv_r, 0), (w16_v, wv_r, 4),
            (w16_v, wv_r, 8), (w16_v, wv_r, 12),
        ]

        # biases replicated across partitions: pre-filled into PSUM by ACT so
        # the PE never runs bias matmuls (start=False accumulates on top).
        # fp16 on the host -> tiny plain HWDGE broadcasts, first on the ring.
        b16_rep = consts.tile([128, 3, TEXT_DIM], F16)
        for i, brow in enumerate((bq, bk, bv)):
            nc.sync.dma_start(
                out=b16_rep[:, i, :],
                in_=bass.AP(
                    tensor=brow[:].tensor, offset=0, ap=[[0, 128], [1, TEXT_DIM]]
                ),
            )

        eps_sb = consts.tile([128, 1], F32)
        nc.vector.memset(eps_sb, 1e-5)
        if apply_affine:
            gamma_rep = consts.tile([128, TEXT_DIM], F16)
            beta_rep = consts.tile([128, TEXT_DIM], F16)
            nc.gpsimd.dma_start(
                out=gamma_rep,
                in_=bass.AP(
                    tensor=gamma[:].tensor, offset=0, ap=[[0, 128], [1, TEXT_DIM]]
                ),
            )
            nc.gpsimd.dma_start(
                out=beta_rep,
                in_=bass.AP(
                    tensor=beta[:].tensor, offset=0, ap=[[0, 128], [1, TEXT_DIM]]
                ),
            )

        # ---------------- 3-stage software pipeline ----------------
        # stage A (iter j):   projections + psum copies + scores + exp
        # stage B (iter j+1): softmax weights + attend + residual add
        # stage C (iter j+2): layernorm + gamma/beta + store
        # This keeps the DVE FIFO free of head-of-line stalls: every DVE op
        # emitted only depends on work issued at least one iteration earlier.

        def project(xt, w16, nchunks, bias_idx, bs):
            # fp8 DoubleRow: one instruction contracts two 128-row k-chunks.
            # Both f-halves are driven from the same stationary X^T pair, with
            # the second matmul marked ldweights=False so the PE streams it
            # from the already-loaded weights (the 2-plane DR stationary load
            # does not hide under the previous stream the way fp16 loads do).
            pt0 = psum.tile([128, 512], F32, tag="psum")
            pt1 = psum.tile([128, 512], F32, tag="psum")
            ps = [pt0, pt1]
            for f in range(2):
                nc.scalar.copy(
                    out=ps[f], in_=b16_rep[:, bias_idx, f * 512 : (f + 1) * 512]
                )
            for c in range(0, nchunks, 2):
                for f in range(2):
                    mm = nc.tensor.matmul(
                        ps[f],
                        lhsT=xt[:, c : c + 2, bs],
                        rhs=w16[:, c : c + 2, f * 512 : (f + 1) * 512],
                        start=False,
                        stop=(c == nchunks - 2),
                        perf_mode=mybir.MatmulPerfMode.DoubleRow,
                    )
                    if f == 1:
                        mm.ins.ldweights = False
            return ps

        # All wide DVE ops process a PAIR of 128-row batch tiles (one slab) in
        # a single instruction — halves the per-op fixed overhead and the
        # semaphore traffic.  PE/ACT/PSUM work stays per 128-row half (psum
        # bank budget and [P,1] ACT scale operands force that).
        HH2 = 2 * H * H

        def stage_a(p, xt_text, xt_img):
            row0 = p * 2 * BT
            text_sb = work.tile([128, 2, TEXT_DIM], F32, tag="text_sb")
            nc.sync.dma_start(
                out=text_sb,
                in_=text[row0 : row0 + 2 * BT, :].rearrange("(s p) f -> p s f", p=128),
            )

            q16 = qkv.tile([128, 2, TEXT_DIM], F16, tag="q16")
            k16 = qkv.tile([128, 2, TEXT_DIM], F16, tag="k16")
            vt16 = qkv.tile([128, 2, TEXT_DIM], F16, tag="vt16")
            unscale = 1.0 / W_SCALE
            for s in range(2):
                bs = slice(s * BT, (s + 1) * BT)
                qp = project(xt_text, w16_q, NTC, 0, bs)
                kp = project(xt_img, w16_k, NIC, 1, bs)
                vp = project(xt_img, w16_v, NIC, 2, bs)
                for dst, srcs in ((q16, qp), (k16, kp), (vt16, vp)):
                    for f in range(2):
                        nc.scalar.activation(
                            out=dst[:, s, f * 512 : (f + 1) * 512],
                            in_=srcs[f],
                            func=mybir.ActivationFunctionType.Identity,
                            scale=unscale,
                        )

            # scores: prod[b, s, h, g, d] = Q[b,s,h,d] * K[b,s,g,d]
            prod = prods.tile([128, 2, H * H * HD], F16, tag="prod")
            scr2 = scr2p.tile([128, 2, H * H * HD // 2], F16, tag="scr2")
            nc.vector.tensor_tensor(
                out=_ap(prod, [[H * H * HD, 2], [H * HD, H], [HD, H], [1, HD]]),
                in0=_ap(q16, [[TEXT_DIM, 2], [128, 8], [0, 8], [1, 128]]),
                in1=_ap(k16, [[TEXT_DIM, 2], [0, 8], [128, 8], [1, 128]]),
                op=mybir.AluOpType.mult,
            )
            # d-reduction: binary TT-add tree over 128 (s,h,g) groups
            cur, nxt = prod, scr2
            d = HD
            while d > 8:
                nc.vector.tensor_tensor(
                    out=_ap(nxt, [[d // 2, HH2], [1, d // 2]]),
                    in0=_ap(cur, [[d, HH2], [1, d // 2]]),
                    in1=bass.AP(tensor=cur.tensor, offset=cur.offset + d // 2,
                                ap=[list(cur.ap[0]), [d, HH2], [1, d // 2]]),
                    op=mybir.AluOpType.add,
                )
                cur, nxt = nxt, cur
                d //= 2
            s16 = small.tile([128, HH2], F16, tag="s16")
            with nc.allow_low_precision("fp16 scores; DVE ALU accumulates fp32"):
                nc.vector.tensor_reduce(
                    out=s16,
                    in_=_ap(cur, [[8, HH2], [1, 8]]),
                    axis=mybir.AxisListType.X,
                    op=mybir.AluOpType.add,
                )
            e16 = small.tile([128, HH2], F16, tag="e16")
            nc.scalar.activation(
                out=e16, in_=s16,
                func=mybir.ActivationFunctionType.Exp,
                scale=float(INV_SQRT_HD),
            )
            return dict(
                p=p, text_sb=text_sb, vt16=vt16, e16=e16, prod=prod, scr2=scr2
            )

        def stage_b1(t):
            # den + 1/den early: the ACT round trip (Ln, Exp) completes while
            # the DVE runs the interleaved bn_stats of the pair two back.
            e16 = t["e16"]
            den = small.tile([128, 2 * H], F32, tag="den")
            nc.vector.tensor_reduce(
                out=den,
                in_=_ap(e16, [[8, 2 * H], [1, 8]]),
                axis=mybir.AxisListType.X,
                op=mybir.AluOpType.add,
            )
            lden = small.tile([128, 2 * H], F32, tag="lden")
            nc.scalar.activation(
                out=lden, in_=den, func=mybir.ActivationFunctionType.Ln
            )
            rden = small.tile([128, 2 * H], F32, tag="rden")
            nc.scalar.activation(
                out=rden, in_=lden, func=mybir.ActivationFunctionType.Exp, scale=-1.0
            )
            t["rden"] = rden

        def stage_b2(t):
            e16, vt16, prod, scr2 = t["e16"], t["vt16"], t["prod"], t["scr2"]
            rden = t["rden"]
            a16 = small.tile([128, HH2], F16, tag="a16")
            nc.vector.tensor_tensor(
                out=_ap(a16, [[8, 2 * H], [1, 8]]),
                in0=_ap(e16, [[8, 2 * H], [1, 8]]),
                in1=_ap(rden, [[1, 2 * H], [0, 8]]),
                op=mybir.AluOpType.mult,
            )
            # attend: prod2[b, s, h, d, g] = A[b,s,h,g] * Vperm[b, s, d*8+g]
            nc.vector.tensor_tensor(
                out=_ap(prod, [[H * H * HD, 2], [H * HD, 8], [8, 128], [1, 8]]),
                in0=_ap(a16, [[H * H, 2], [8, 8], [0, 128], [1, 8]]),
                in1=_ap(vt16, [[TEXT_DIM, 2], [0, 8], [8, 128], [1, 8]]),
                op=mybir.AluOpType.mult,
            )
            # g-reduction: dense-output TT tree over 2048 (s,h,d) groups
            GR = 2 * H * HD
            nc.vector.tensor_tensor(
                out=_ap(scr2, [[4, GR], [1, 4]]),
                in0=_ap(prod, [[8, GR], [1, 4]]),
                in1=bass.AP(tensor=prod.tensor, offset=prod.offset + 4,
                            ap=[list(prod.ap[0]), [8, GR], [1, 4]]),
                op=mybir.AluOpType.add,
            )
            nc.vector.tensor_tensor(
                out=_ap(prod, [[2, GR], [1, 2]]),
                in0=_ap(scr2, [[4, GR], [1, 2]]),
                in1=bass.AP(tensor=scr2.tensor, offset=scr2.offset + 2,
                            ap=[list(scr2.ap[0]), [4, GR], [1, 2]]),
                op=mybir.AluOpType.add,
            )
            att16 = work.tile([128, 2, TEXT_DIM], F16, tag="att16")
            nc.vector.tensor_tensor(
                out=att16,
                in0=_ap(prod, [[2, GR]]),
                in1=bass.AP(tensor=prod.tensor, offset=prod.offset + 1,
                            ap=[list(prod.ap[0]), [2, GR]]),
                op=mybir.AluOpType.add,
            )
            # residual on the otherwise-idle Pool engine (the SWDGE accum-DMA
            # alternative trickles 4KB RMW packets at ~13GB/s and stalls LN)
            x = work.tile([128, 2, TEXT_DIM], F32, tag="x")
            nc.gpsimd.tensor_tensor(
                out=x, in0=t["text_sb"], in1=att16, op=mybir.AluOpType.add
            )
            t["x"] = x

        def stage_c1(t):
            x = t["x"]
            stats = small.tile([128, 2, 2, 6], F32, tag="stats")
            mv = small.tile([128, 2, 2], F32, tag="mv")
            for s in range(2):
                nc.vector.bn_stats(out=stats[:, s, 0, :], in_=x[:, s, 0:512])
                nc.vector.bn_stats(out=stats[:, s, 1, :], in_=x[:, s, 512:1024])
            for s in range(2):
                nc.vector.bn_aggr(out=mv[:, s, :], in_=stats[:, s, :, :])
            t["mv"] = mv

        def stage_c2(t):
            x, mv = t["x"], t["mv"]
            row0 = t["p"] * 2 * BT
            y32 = outs.tile([128, 2, TEXT_DIM], F32, tag="y32")
            for s in range(2):
                # rs = 1/sqrt(var+eps) = exp(-0.5*ln(var+eps)); Ln and Exp
                # live in the same ACT table, Sqrt does not.
                lnv = small.tile([128, 1], F32, tag="lnv")
                nc.scalar.activation(
                    out=lnv, in_=mv[:, s, 1:2],
                    func=mybir.ActivationFunctionType.Ln,
                    bias=eps_sb, scale=1.0,
                )
                rs = small.tile([128, 1], F32, tag="rs")
                nc.scalar.activation(
                    out=rs, in_=lnv,
                    func=mybir.ActivationFunctionType.Exp,
                    scale=-0.5,
                )
                nmr = small.tile([128, 1], F32, tag="nmr")
                nc.gpsimd.tensor_scalar(
                    out=nmr, in0=mv[:, s, 0:1],
                    scalar1=rs, scalar2=-1.0,
                    op0=mybir.AluOpType.mult, op1=mybir.AluOpType.mult,
                )
                if apply_affine:
                    xn16 = work.tile([128, TEXT_DIM], F16, tag="xn16")
                    nc.scalar.activation(
                        out=xn16, in_=x[:, s, :],
                        func=mybir.ActivationFunctionType.Identity,
                        scale=rs, bias=nmr,
                    )
                    nc.gpsimd.tensor_tensor(
                        out=xn16, in0=xn16, in1=gamma_rep, op=mybir.AluOpType.mult
                    )
                    y16 = outs.tile([128, TEXT_DIM], F16, tag="y16")
                    nc.gpsimd.tensor_tensor(
                        out=y16, in0=xn16, in1=beta_rep, op=mybir.AluOpType.add
                    )
                    nc.gpsimd.dma_start(
                        out=y[row0 + s * BT : row0 + (s + 1) * BT, :], in_=y16
                    )
                else:
                    # gamma==1 / beta==0 fast path: the ACT normalize writes
                    # the fp32 output pair tile directly
                    nc.scalar.activation(
                        out=y32[:, s, :], in_=x[:, s, :],
                        func=mybir.ActivationFunctionType.Identity,
                        scale=rs, bias=nmr,
                    )
            if not apply_affine:
                nc.sync.dma_start(
                    out=y[row0 : row0 + 2 * BT, :].rearrange(
                        "(s p) f -> p s f", p=128
                    ),
                    in_=y32,
                )

        pend = []
        n_phases = (nt + phase - 1) // phase
        for ph in range(n_phases):
            b0 = ph * bw
            # slabs are plain fp8 loads -> HWDGE (sync); issuing them on the
            # sync ring ahead of the weight burst makes phase-0 land first,
            # and keeps the rings uncongested for text/y traffic
            xt_text = slabs.tile([128, NTC, bw], F8, tag="xt_text")
            xt_img = slabs.tile([128, NIC, bw], F8, tag="xt_img")
            nc.sync.dma_start(
                out=xt_text,
                in_=text_t[:, b0 : b0 + bw].rearrange("(c p) b -> p c b", p=128),
            )
            nc.sync.dma_start(
                out=xt_img,
                in_=image_t[:, b0 : b0 + bw].rearrange("(c p) b -> p c b", p=128),
            )
            if ph == 0:
                # weights after the first slab so tile 0 lhsT lands first
                for w16, wr, c0 in wload:
                    nc.sync.dma_start(
                        out=w16[:, c0 : c0 + 4, :], in_=wr[:, c0 : c0 + 4, :]
                    )

            # Emission order per pair-iteration: b1(p-1) kicks the den->rden
            # ACT round trip, c1(p-2) bn_stats fill the DVE while it lands,
            # b2(p-1) attends (freeing prod before stage_a(p) reuses the
            # single-buffered pair tile), then stage_a(p) runs the scores.
            if pend:
                stage_b1(pend[-1])
            if len(pend) >= 2:
                stage_c1(pend[-2])
            if pend:
                stage_b2(pend[-1])
            cur = stage_a(ph, xt_text, xt_img)
            if len(pend) >= 2:
                stage_c2(pend[-2])
            pend.append(cur)
        stage_b1(pend[-1])
        stage_c1(pend[-2])
        stage_b2(pend[-1])
        stage_c2(pend[-2])
        stage_c1(pend[-1])
        stage_c2(pend[-1])


@functools.lru_cache(maxsize=2)
def _built(b_loc: int, apply_affine: bool = False):
    return build_bass(b_loc, apply_affine)


def _shard_inputs(inputs: dict, b_loc: int, n_cores: int):
    import ml_dtypes

    f32 = lambda a: np.ascontiguousarray(np.asarray(a), dtype=np.float32)
    text = f32(inputs["text_features"])
    image = f32(inputs["image_features"])
    # fp8 e4m3 quantization: activations unscaled (|x|<6), weights pre-scaled
    # by W_SCALE to stay clear of e4m3 subnormals; TRN e4m3 max is ±240.
    f8 = lambda a: np.ascontiguousarray(
        np.clip(np.asarray(a, np.float32), -240, 240).astype(ml_dtypes.float8_e4m3fn)
    )
    ws = np.float32(W_SCALE)
    wq_t = f8(np.asarray(inputs["Wq"], np.float32).T * ws)
    wk_t = f8(np.asarray(inputs["Wk"], np.float32).T * ws)
    # V output features permuted to the [d2][g][d1] attend layout
    wv_t = f8(np.asarray(inputs["Wv"], np.float32).T[:, V_PERM] * ws)
    row = lambda a: f32(a).reshape(1, -1)
    row16 = lambda a: np.ascontiguousarray(a, dtype=np.float16)
    bq, bk = row16(row(inputs["bq"]) * ws), row16(row(inputs["bk"]) * ws)
    bv = row16(row(np.asarray(inputs["bv"])[V_PERM]) * ws)
    gm, bt = row(inputs["gamma"]), row(inputs["beta"])

    in_maps = []
    for c in range(n_cores):
        sl = slice(c * b_loc, (c + 1) * b_loc)
        in_maps.append(
            {
                "text_t": f8(text[sl].T),
                "image_t": f8(image[sl].T),
                "text": f32(text[sl]),
                "wq_t": wq_t,
                "wk_t": wk_t,
                "wv_t": wv_t,
                "bq": bq,
                "bk": bk,
                "bv": bv,
                "gamma": gm,
                "beta": bt,
            }
        )
    return in_maps


def kernel(**inputs) -> np.ndarray:
    # LN affine folds to identity when gamma==1 and beta==0 (true for this
    # problem); otherwise fall back to the variant that applies it on-chip.
    affine = not (
        np.allclose(np.asarray(inputs["gamma"]), 1.0)
        and np.allclose(np.asarray(inputs["beta"]), 0.0)
    )
    nc = _built(B_LOC, affine)
    in_maps = _shard_inputs(inputs, B_LOC, N_CORES)
    res = bass_utils.run_bass_kernel_spmd(nc, in_maps, core_ids=list(range(N_CORES)))
    return np.concatenate([r["y"] for r in res.results], axis=0)



# revision 45
# speedup vs baseline: 1.1155x; 1.1155x over previous
"""
MultiHeadCrossAttention Trainium2 kernel (Bass/Tile), data-parallel over batch
on 8 NeuronCores.

Reference computation (per batch row b):
    Q = text @ Wq.T + bq          [B, 1024] -> [B, 8, 128]
    K = image @ Wk.T + bk         [B, 1024] -> [B, 8, 128]
    V = image @ Wv.T + bv         [B, 1024] -> [B, 8, 128]
    scores[b,h,g] = Q[b,h,:].K[b,g,:] / sqrt(128)
    attn = softmax_g(scores)
    attended[b,h,:] = sum_g attn[b,h,g] V[b,g,:]
    y = LayerNorm(text + attended) * gamma + beta

Design (per core, B_loc = 2048 batch rows, 16 tiles of 128):
  - Projections on the PE in fp8 e4m3 DoubleRow mode (2 k-chunks per
    instruction, 2x fp16 throughput; fp32 PSUM accumulation).  Stationary
    operand = X^T batch tile [128 d x 2 x 128 b] shared across both 512-wide
    f-halves (ldweights=False on the second), moving operand = W^T chunks.
    X quantized to e4m3 on the host unscaled; W scaled by 4096 to dodge e4m3
    subnormals, un-scaled in the ACT PSUM->SBUF copies.  Biases pre-filled
    into PSUM by ACT broadcast copies (no bias matmuls; start=False).
  - Attention entirely in batch-on-partition layout on the DVE (fp16, 2
    elem/cycle): scores product + binary TT add-tree over d, ACT exp, softmax
    denominator reciprocal via ACT exp(-ln(den)), attend product against the
    host-permuted V layout + TT tree over g.
  - Residual add on the Pool engine; LayerNorm via bn_stats/bn_aggr,
    rs=exp(-0.5 ln(var+eps)) on ACT.  gamma==1/beta==0 for this problem, so
    the affine collapses into the ACT normalize (checked on the host with an
    on-chip fallback variant that applies it).
  - Pipeline emission order per iteration: b1(j-1) den/rden, a(j) scores
    (fills the ACT round trips), b2(j-1) attend+residual, c(j-2) LN+store.
"""

import functools
import sys

import numpy as np

sys.path.insert(0, "/opt/trn_rl_repo")

import concourse.bass as bass  # noqa: E402
import concourse.tile as tile  # noqa: E402
from concourse import bacc, bass_utils, mybir  # noqa: E402


def _patch_act_tables():
    """Force every activation we use (Exp/Ln/Square/Copy/Identity) to resolve
    to the one table set that holds them all (natural_log_exp_and_others), so
    bacc emits a single ACT table load instead of thrashing (1.28us/swap).
    Only the chooser's view is edited; set ids stay positional, so walrus
    still maps to the real act_info.json tables."""
    import concourse.hw_specs as hw_specs

    orig = hw_specs.get_activation_tables
    if getattr(orig, "_mhca_patched", False):
        return

    A = mybir.ActivationFunctionType
    KEEP = "natural_log_exp_and_others"

    @functools.cache
    def patched(arch):
        tabs = {k: set(v) for k, v in orig(arch).items()}
        for k, s in tabs.items():
            if k != KEEP:
                for f in (A.Exp, A.Ln, A.Square, A.Copy, A.Identity):
                    s.discard(f)
        return tabs

    patched._mhca_patched = True
    hw_specs.get_activation_tables = patched
    import concourse.bass_interp as _bi

    _bi.get_activation_tables = patched
    bacc.get_activation_tables = patched


_patch_act_tables()

# Problem constants (hardcoded per contest contract)
B = 16384
N_CORES = 8
B_LOC = B // N_CORES  # 2048
TEXT_DIM = 1024
IMAGE_DIM = 2048
H = 8
HD = 128
NTC = TEXT_DIM // 128  # 8 text d-chunks
NIC = IMAGE_DIM // 128  # 16 image d-chunks

BT = 128  # batch tile (partition dim)
NT = B_LOC // BT  # 16 batch tiles per core
PHASE = 2  # batch tiles per X^T slab load
BW = PHASE * BT  # slab batch width

F16 = mybir.dt.float16
F32 = mybir.dt.float32
F8 = mybir.dt.float8e4

INV_SQRT_HD = 1.0 / np.sqrt(128.0)
W_SCALE = 4096.0  # fp8 weight pre-scale (keeps w out of e4m3 subnormals)

# V feature permutation: f' = d*8 + g for original f = g*128 + d, i.e. V is
# stored with the 8 head values of each hidden position adjacent, so the
# attend product / g-reduction reads contiguous 8-element runs.
_d, _g = np.meshgrid(np.arange(128), np.arange(8), indexing="ij")
V_PERM = (_g * 128 + _d).reshape(-1)  # V_PERM[f'] = original f


def build_bass(b_loc: int = B_LOC, apply_affine: bool = False) -> bass.Bass:
    nt = b_loc // BT
    phase = min(PHASE, nt)
    bw = phase * BT

    nc = bacc.Bacc(trn_type="TRN2", debug=False, name="mhca_dp", num_swdge_queues=4)

    # ---- DRAM I/O ----
    text_t = nc.dram_tensor("text_t", [TEXT_DIM, b_loc], F8, kind="ExternalInput")
    image_t = nc.dram_tensor("image_t", [IMAGE_DIM, b_loc], F8, kind="ExternalInput")
    text = nc.dram_tensor("text", [b_loc, TEXT_DIM], F32, kind="ExternalInput")
    wq_t = nc.dram_tensor("wq_t", [TEXT_DIM, TEXT_DIM], F8, kind="ExternalInput")
    wk_t = nc.dram_tensor("wk_t", [IMAGE_DIM, TEXT_DIM], F8, kind="ExternalInput")
    wv_t = nc.dram_tensor("wv_t", [IMAGE_DIM, TEXT_DIM], F8, kind="ExternalInput")
    bq = nc.dram_tensor("bq", [1, TEXT_DIM], F16, kind="ExternalInput")
    bk = nc.dram_tensor("bk", [1, TEXT_DIM], F16, kind="ExternalInput")
    bv = nc.dram_tensor("bv", [1, TEXT_DIM], F16, kind="ExternalInput")
    gamma = nc.dram_tensor("gamma", [1, TEXT_DIM], F32, kind="ExternalInput")
    beta = nc.dram_tensor("beta", [1, TEXT_DIM], F32, kind="ExternalInput")
    y = nc.dram_tensor("y", [b_loc, TEXT_DIM], F32, kind="ExternalOutput")

    with tile.TileContext(nc) as tc:
        _body(nc, tc, locals(), nt=nt, phase=phase, bw=bw, apply_affine=apply_affine)
    nc.compile()
    return nc


def _ap(t: bass.AP, dims) -> bass.AP:
    """Raw AP on an SBUF tile: keep its partition dim, custom free dims."""
    return bass.AP(tensor=t.tensor, offset=t.offset, ap=[list(t.ap[0])] + [list(d) for d in dims])


def _body(
    nc: bass.Bass,
    tc: tile.TileContext,
    io: dict,
    *,
    nt: int,
    phase: int,
    bw: int,
    apply_affine: bool,
):
    text_t, image_t, text = io["text_t"], io["image_t"], io["text"]
    wq_t, wk_t, wv_t = io["wq_t"], io["wk_t"], io["wv_t"]
    bq, bk, bv = io["bq"], io["bk"], io["bv"]
    gamma, beta, y = io["gamma"], io["beta"], io["y"]

    import contextlib

    ctx = contextlib.ExitStack()
    with ctx:
        consts = ctx.enter_context(tc.tile_pool(name="consts", bufs=1))
        slabs = ctx.enter_context(tc.tile_pool(name="slabs", bufs=3))
        qkv = ctx.enter_context(tc.tile_pool(name="qkv", bufs=3))
        work = ctx.enter_context(tc.tile_pool(name="work", bufs=2))
        # 2 bufs: scores-prod of tile j is emitted before the attend of tile
        # j-1 (which reuses its prod as workspace) — same buffer would be a
        # write-after-read clobber
        prods = ctx.enter_context(tc.tile_pool(name="prods", bufs=2))
        scr2p = ctx.enter_context(tc.tile_pool(name="scr2p", bufs=2))
        outs = ctx.enter_context(tc.tile_pool(name="outs", bufs=2))
        small = ctx.enter_context(tc.tile_pool(name="small", bufs=3))
        psum = ctx.enter_context(tc.tile_pool(name="psum", bufs=8, space="PSUM"))

        # ---- constants / weights (fp8, host-quantized ×W_SCALE) ----
        w16_q = consts.tile([128, NTC, TEXT_DIM], F8)
        w16_k = consts.tile([128, NIC, TEXT_DIM], F8)
        w16_v = consts.tile([128, NIC, TEXT_DIM], F8)
        # chunked weight loads, interleaved in tile-0 consumption order so the
        # first matmuls can start long before all 20MB of weights land
        wq_r = wq_t[:].rearrange("(c p) f -> p c f", p=128)
        wk_r = wk_t[:].rearrange("(c p) f -> p c f", p=128)
        wv_r = wv_t[:].rearrange("(c p) f -> p c f", p=128)
        wload = [
            (w16_q, wq_r, 0), (w16_k, wk_r, 0),
            (w16_q, wq_r, 4), (w16_k, wk_r, 4),
            (w16_k, wk_r, 8), (w16_k, wk_r, 12),
            (w16_v, wv_r, 0), (w16_v, wv_r, 4),
            (w16_v, wv_r, 8), (w16_v, wv_r, 12),
        ]

        # biases replicated across partitions: pre-filled into PSUM by ACT so
        # the PE never runs bias matmuls (start=False accumulates on top).
        # fp16 on the host -> tiny plain HWDGE broadcasts, first on the ring.
        b16_rep = consts.tile([128, 3, TEXT_DIM], F16)
        for i, brow in enumerate((bq, bk, bv)):
            nc.sync.dma_start(
                out=b16_rep[:, i, :],
                in_=bass.AP(
                    tensor=brow[:].tensor, offset=0, ap=[[0, 128], [1, TEXT_DIM]]
                ),
            )

        eps_sb = consts.tile([128, 1], F32)
        nc.vector.memset(eps_sb, 1e-5)
        if apply_affine:
            gamma_rep = consts.tile([128, TEXT_DIM], F16)
            beta_rep = consts.tile([128, TEXT_DIM], F16)
            nc.gpsimd.dma_start(
                out=gamma_rep,
                in_=bass.AP(
                    tensor=gamma[:].tensor, offset=0, ap=[[0, 128], [1, TEXT_DIM]]
                ),
            )
            nc.gpsimd.dma_start(
                out=beta_rep,
                in_=bass.AP(
                    tensor=beta[:].tensor, offset=0, ap=[[0, 128], [1, TEXT_DIM]]
                ),
            )

        # ---------------- 3-stage software pipeline ----------------
        # stage A (iter j):   projections + psum copies + scores + exp
        # stage B (iter j+1): softmax weights + attend + residual add
        # stage C (iter j+2): layernorm + gamma/beta + store
        # This keeps the DVE FIFO free of head-of-line stalls: every DVE op
        # emitted only depends on work issued at least one iteration earlier.

        def project(xt, w16, nchunks, bias_idx, bs):
            # fp8 DoubleRow: one instruction contracts two 128-row k-chunks.
            # Both f-halves are driven from the same stationary X^T pair, with
            # the second matmul marked ldweights=False so the PE streams it
            # from the already-loaded weights (the 2-plane DR stationary load
            # does not hide under the previous stream the way fp16 loads do).
            pt0 = psum.tile([128, 512], F32, tag="psum")
            pt1 = psum.tile([128, 512], F32, tag="psum")
            ps = [pt0, pt1]
            for f in range(2):
                nc.scalar.copy(
                    out=ps[f], in_=b16_rep[:, bias_idx, f * 512 : (f + 1) * 512]
                )
            for c in range(0, nchunks, 2):
                for f in range(2):
                    mm = nc.tensor.matmul(
                        ps[f],
                        lhsT=xt[:, c : c + 2, bs],
                        rhs=w16[:, c : c + 2, f * 512 : (f + 1) * 512],
                        start=False,
                        stop=(c == nchunks - 2),
                        perf_mode=mybir.MatmulPerfMode.DoubleRow,
                    )
                    if f == 1:
                        mm.ins.ldweights = False
            return ps

        def stage_a(it, xt_text, xt_img, bs):
            row0 = it * BT
            text_sb = work.tile([128, TEXT_DIM], F32, tag="text_sb")
            nc.sync.dma_start(out=text_sb, in_=text[row0 : row0 + BT, :])

            qp = project(xt_text, w16_q, NTC, 0, bs)
            kp = project(xt_img, w16_k, NIC, 1, bs)
            vp = project(xt_img, w16_v, NIC, 2, bs)

            # PSUM -> SBUF fp16 copies (ACT).  Wv/bv are host-permuted to the
            # [d2][g][d1] attend layout, so all copies are contiguous.
            q16 = qkv.tile([128, TEXT_DIM], F16, tag="q16")
            k16 = qkv.tile([128, TEXT_DIM], F16, tag="k16")
            vt16 = qkv.tile([128, TEXT_DIM], F16, tag="vt16")
            unscale = 1.0 / W_SCALE
            for dst, srcs in ((q16, qp), (k16, kp), (vt16, vp)):
                for f in range(2):
                    nc.scalar.activation(
                        out=dst[:, f * 512 : (f + 1) * 512],
                        in_=srcs[f],
                        func=mybir.ActivationFunctionType.Identity,
                        scale=unscale,
                    )

            # scores: prod[b, h, g, d] = Q[b,h,d] * K[b,g,d]
            prod = prods.tile([128, H * H * HD], F16, tag="prod")
            scr2 = scr2p.tile([128, H * H * HD // 2], F16, tag="scr2")
            nc.vector.tensor_tensor(
                out=prod[:].rearrange("p (h g d) -> p h g d", h=H, g=H),
                in0=_ap(q16, [[128, 8], [0, 8], [1, 128]]),
                in1=_ap(k16, [[0, 8], [128, 8], [1, 128]]),
                op=mybir.AluOpType.mult,
            )
            # d-reduction: binary TT-add tree with dense (compacted) outputs
            # ping-ponging between prod and scr2 — segmented tensor_reduce
            # runs at 1 elem/cycle on cayman, the fp16 TT tree at 2 —
            # then one tensor_reduce of the remaining 8.
            cur, nxt = prod, scr2
            d = HD
            while d > 8:
                nc.vector.tensor_tensor(
                    out=_ap(nxt, [[d // 2, H * H], [1, d // 2]]),
                    in0=_ap(cur, [[d, H * H], [1, d // 2]]),
                    in1=bass.AP(tensor=cur.tensor, offset=cur.offset + d // 2,
                                ap=[list(cur.ap[0]), [d, H * H], [1, d // 2]]),
                    op=mybir.AluOpType.add,
                )
                cur, nxt = nxt, cur
                d //= 2
            s16 = small.tile([128, H * H], F16, tag="s16")
            with nc.allow_low_precision("fp16 scores; DVE ALU accumulates fp32"):
                nc.vector.tensor_reduce(
                    out=s16,
                    in_=_ap(cur, [[8, H * H], [1, 8]]),
                    axis=mybir.AxisListType.X,
                    op=mybir.AluOpType.add,
                )
            e16 = small.tile([128, H * H], F16, tag="e16")
            nc.scalar.activation(
                out=e16, in_=s16,
                func=mybir.ActivationFunctionType.Exp,
                scale=float(INV_SQRT_HD),
            )
            return dict(
                it=it, text_sb=text_sb, vt16=vt16, e16=e16, prod=prod, scr2=scr2
            )

        def stage_b1(t):
            # den + 1/den early: the ACT round trip (Ln, Exp) completes while
            # the DVE runs the interleaved bn_stats of the tile two back.
            e16 = t["e16"]
            den = small.tile([128, H], F32, tag="den")
            nc.vector.tensor_reduce(
                out=den,
                in_=e16[:].rearrange("p (h g) -> p h g", h=H),
                axis=mybir.AxisListType.X,
                op=mybir.AluOpType.add,
            )
            lden = small.tile([128, H], F32, tag="lden")
            nc.scalar.activation(
                out=lden, in_=den, func=mybir.ActivationFunctionType.Ln
            )
            rden = small.tile([128, H], F32, tag="rden")
            nc.scalar.activation(
                out=rden, in_=lden, func=mybir.ActivationFunctionType.Exp, scale=-1.0
            )
            t["rden"] = rden

        def stage_b2(t):
            e16, vt16, prod, scr2 = t["e16"], t["vt16"], t["prod"], t["scr2"]
            rden = t["rden"]
            a16 = small.tile([128, H * H], F16, tag="a16")
            nc.vector.tensor_tensor(
                out=a16[:].rearrange("p (h g) -> p h g", h=H),
                in0=e16[:].rearrange("p (h g) -> p h g", h=H),
                in1=_ap(rden, [[1, 8], [0, 8]]),
                op=mybir.AluOpType.mult,
            )
            # attend: prod2[b, h, d, g] = A[b,h,g] * Vperm[b, d*8+g]
            nc.vector.tensor_tensor(
                out=prod[:].rearrange("p (h d g) -> p h d g", h=H, d=HD),
                in0=_ap(a16, [[8, 8], [0, 128], [1, 8]]),
                in1=_ap(vt16, [[0, 8], [8, 128], [1, 8]]),
                op=mybir.AluOpType.mult,
            )
            # g-reduction: dense-output TT tree prod -> scr2 -> prod, final
            # level writes fp32 attended into x; the residual text rows are
            # then accumulated on top by the SWDGE add-DMA.
            nc.vector.tensor_tensor(
                out=_ap(scr2, [[4, H * HD], [1, 4]]),
                in0=_ap(prod, [[8, H * HD], [1, 4]]),
                in1=bass.AP(tensor=prod.tensor, offset=prod.offset + 4,
                            ap=[list(prod.ap[0]), [8, H * HD], [1, 4]]),
                op=mybir.AluOpType.add,
            )
            nc.vector.tensor_tensor(
                out=_ap(prod, [[2, H * HD], [1, 2]]),
                in0=_ap(scr2, [[4, H * HD], [1, 2]]),
                in1=bass.AP(tensor=scr2.tensor, offset=scr2.offset + 2,
                            ap=[list(scr2.ap[0]), [4, H * HD], [1, 2]]),
                op=mybir.AluOpType.add,
            )
            att16 = work.tile([128, TEXT_DIM], F16, tag="att16")
            nc.vector.tensor_tensor(
                out=att16,
                in0=_ap(prod, [[2, H * HD]]),
                in1=bass.AP(tensor=prod.tensor, offset=prod.offset + 1,
                            ap=[list(prod.ap[0]), [2, H * HD]]),
                op=mybir.AluOpType.add,
            )
            # residual on the otherwise-idle Pool engine (the SWDGE accum-DMA
            # alternative trickles 4KB RMW packets at ~13GB/s and stalls LN)
            x = work.tile([128, TEXT_DIM], F32, tag="x")
            nc.gpsimd.tensor_tensor(
                out=x, in0=t["text_sb"], in1=att16, op=mybir.AluOpType.add
            )
            t["x"] = x

        def stage_c1(t):
            x = t["x"]
            stats = small.tile([128, 2, 6], F32, tag="stats")
            nc.vector.bn_stats(out=stats[:, 0, :], in_=x[:, 0:512])
            nc.vector.bn_stats(out=stats[:, 1, :], in_=x[:, 512:1024])
            mv = small.tile([128, 2], F32, tag="mv")
            nc.vector.bn_aggr(out=mv, in_=stats)
            t["mv"] = mv

        def stage_c2(t):
            x, mv = t["x"], t["mv"]
            row0 = t["it"] * BT
            # rs = 1/sqrt(var+eps) = exp(-0.5*ln(var+eps)); Ln and Exp live in
            # the same ACT table (natural_log_exp_and_others), Sqrt does not.
            lnv = small.tile([128, 1], F32, tag="lnv")
            nc.scalar.activation(
                out=lnv, in_=mv[:, 1:2],
                func=mybir.ActivationFunctionType.Ln,
                bias=eps_sb, scale=1.0,
            )
            rs = small.tile([128, 1], F32, tag="rs")
            nc.scalar.activation(
                out=rs, in_=lnv,
                func=mybir.ActivationFunctionType.Exp,
                scale=-0.5,
            )
            nmr = small.tile([128, 1], F32, tag="nmr")
            nc.gpsimd.tensor_scalar(
                out=nmr, in0=mv[:, 0:1],
                scalar1=rs, scalar2=-1.0,
                op0=mybir.AluOpType.mult, op1=mybir.AluOpType.mult,
            )
            if apply_affine:
                xn16 = work.tile([128, TEXT_DIM], F16, tag="xn16")
                nc.scalar.activation(
                    out=xn16, in_=x,
                    func=mybir.ActivationFunctionType.Identity,
                    scale=rs, bias=nmr,
                )
                nc.gpsimd.tensor_tensor(
                    out=xn16, in0=xn16, in1=gamma_rep, op=mybir.AluOpType.mult
                )
                y16 = outs.tile([128, TEXT_DIM], F16, tag="y16")
                nc.gpsimd.tensor_tensor(
                    out=y16, in0=xn16, in1=beta_rep, op=mybir.AluOpType.add
                )
                nc.gpsimd.dma_start(out=y[row0 : row0 + BT, :], in_=y16)
            else:
                # gamma==1 / beta==0 fast path: LN affine is the identity, so
                # the ACT normalize writes the fp32 output tile directly
                y32 = outs.tile([128, TEXT_DIM], F32, tag="y32")
                nc.scalar.activation(
                    out=y32, in_=x,
                    func=mybir.ActivationFunctionType.Identity,
                    scale=rs, bias=nmr,
                )
                nc.sync.dma_start(out=y[row0 : row0 + BT, :], in_=y32)

        pend = []
        n_phases = (nt + phase - 1) // phase
        for ph in range(n_phases):
            b0 = ph * bw
            # slabs are plain fp8 loads -> HWDGE (sync); issuing them on the
            # sync ring ahead of the weight burst makes phase-0 land first,
            # and keeps SWDGE free for the per-tile x+=text accum DMAs
            xt_text = slabs.tile([128, NTC, bw], F8, tag="xt_text")
            xt_img = slabs.tile([128, NIC, bw], F8, tag="xt_img")
            nc.sync.dma_start(
                out=xt_text,
                in_=text_t[:, b0 : b0 + bw].rearrange("(c p) b -> p c b", p=128),
            )
            nc.sync.dma_start(
                out=xt_img,
                in_=image_t[:, b0 : b0 + bw].rearrange("(c p) b -> p c b", p=128),
            )
            if ph == 0:
                # weights after the first slab so tile 0 lhsT lands first;
                # fp16 in DRAM so these go over HWDGE (no cast needed)
                for w16, wr, c0 in wload:
                    nc.sync.dma_start(
                        out=w16[:, c0 : c0 + 4, :], in_=wr[:, c0 : c0 + 4, :]
                    )

            for j in range(phase):
                it = ph * phase + j
                if it >= nt:
                    break
                # Emission order per iteration: b1(j-1) kicks the den->rden
                # ACT round trip, stage_a(j)'s scores+tree fill the DVE while
                # it (and exp(j)) land, b2(j-1) runs the attend, c1/c2(j-2)
                # close out LN.  Every DVE op then has its ACT inputs ready.
                if pend:
                    stage_b1(pend[-1])
                cur = stage_a(it, xt_text, xt_img, slice(j * BT, (j + 1) * BT))
                if pend:
                    stage_b2(pend[-1])
                if len(pend) >= 2:
                    stage_c1(pend[-2])
                    stage_c2(pend[-2])
                pend.append(cur)
        stage_b1(pend[-1])
        stage_b2(pend[-1])
        stage_c1(pend[-2])
        stage_c2(pend[-2])
        stage_c1(pend[-1])
        stage_c2(pend[-1])


@functools.lru_cache(maxsize=2)
def _built(b_loc: int, apply_affine: bool = False):
    return build_bass(b_loc, apply_affine)


def _shard_inputs(inputs: dict, b_loc: int, n_cores: int):
    import ml_dtypes

    f32 = lambda a: np.ascontiguousarray(np.asarray(a), dtype=np.float32)
    text = f32(inputs["text_features"])
    image = f32(inputs["image_features"])
    # fp8 e4m3 quantization: activations unscaled (|x|<6), weights pre-scaled
    # by W_SCALE to stay clear of e4m3 subnormals; TRN e4m3 max is ±240.
    f8 = lambda a: np.ascontiguousarray(
        np.clip(np.asarray(a, np.float32), -240, 240).astype(ml_dtypes.float8_e4m3fn)
    )
    ws = np.float32(W_SCALE)
    wq_t = f8(np.asarray(inputs["Wq"], np.float32).T * ws)
    wk_t = f8(np.asarray(inputs["Wk"], np.float32).T * ws)
    # V output features permuted to the [d2][g][d1] attend layout
    wv_t = f8(np.asarray(inputs["Wv"], np.float32).T[:, V_PERM] * ws)
    row = lambda a: f32(a).reshape(1, -1)
    row16 = lambda a: np.ascontiguousarray(a, dtype=np.float16)
    bq, bk = row16(row(inputs["bq"]) * ws), row16(row(inputs["bk"]) * ws)
    bv = row16(row(np.asarray(inputs["bv"])[V_PERM]) * ws)
    gm, bt = row(inputs["gamma"]), row(inputs["beta"])

    in_maps = []
    for c in range(n_cores):
        sl = slice(c * b_loc, (c + 1) * b_loc)
        in_maps.append(
            {
                "text_t": f8(text[sl].T),
                "image_t": f8(image[sl].T),
                "text": f32(text[sl]),
                "wq_t": wq_t,
                "wk_t": wk_t,
                "wv_t": wv_t,
                "bq": bq,
                "bk": bk,
                "bv": bv,
                "gamma": gm,
                "beta": bt,
            }
        )
    return in_maps


def kernel(**inputs) -> np.ndarray:
    # LN affine folds to identity when gamma==1 and beta==0 (true for this
    # problem); otherwise fall back to the variant that applies it on-chip.
    affine = not (
        np.allclose(np.asarray(inputs["gamma"]), 1.0)
        and np.allclose(np.asarray(inputs["beta"]), 0.0)
    )
    nc = _built(B_LOC, affine)
    in_maps = _shard_inputs(inputs, B_LOC, N_CORES)
    res = bass_utils.run_bass_kernel_spmd(nc, in_maps, core_ids=list(range(N_CORES)))
    return np.concatenate([r["y"] for r in res.results], axis=0)

